# revision 8
# baseline (speedup 1.0000x reference)
"""VQ codebook-lookup kernel for TRN2, data-parallel over batch on 8 NeuronCores.

Reference computation (per batch b with class c[b]):
  z = z_e_x[b] viewed as [N=4096, D=256] (D innermost)
  cb = embedding[c[b]*512:(c[b]+1)*512]            # [K=512, D]
  idx[n] = argmin_k ||z[n] - cb[k]||^2 = argmax_k (z[n].cb[k] - ||cb[k]||^2/2)
  out[n] = cb[idx[n]]

Device strategy per core (4 batches), engines balanced:
  - scores S[n,k] on TensorE: fp32r matmuls with the codebook split into
    bf16-exact halves (c1=bf16(ct), c2=ct-c1) -- kills fp32r's moving-side
    rounding, ~1.5e-4 worst-case dot error; bias folded in as a
    contraction-1 ones x bias matmul so PSUM holds biased scores directly
  - ScalarE moves biased scores PSUM->SBUF
  - VectorE: row-max, then ONE fused scalar_tensor_tensor computes
    idx[n] = sum_k (S==mx)*k, plus a tiny cast to uint32
  - GPSIMD issues an indirect DMA per n-tile that gathers the selected
    codebook rows (bf16) straight from DRAM -- no transpose, no one-hot
    matmul, no PSUM round-trip for the gather
Host side only reindexes/splits operands and reassembles the output.
"""

import sys

sys.path.insert(0, "/opt/trn_rl_repo")

import numpy as np

B, D, HH, WW = 32, 256, 64, 64
N = HH * WW            # 4096 positions per batch
K = 512                # codes per class
NUM_CLASSES = 60
NCORES = 8
BPC = B // NCORES      # batches per core
NT = N // 128          # 32 n-tiles per batch

DIST = "f32r2"         # "f32r2" (fp32r c-split 2-pass) | "bf3" (split-bf16 3-pass)

_CACHE = {}

# set by test harness to request an NTFF profile
TRACE = False
LAST_EXEC_NS = None


def _build(bpc=BPC, nt=NT, repeat=1, dist=None, tail="full", psum_bufs=3,
           sco_bufs=3, zb_bufs=2):
    # tail: "full" | "nored" (PE only) | "noidx" (+Act copy, DVE max)
    #       | "nodma" (+stt idx extract; skip the indirect gather)
    from concourse import bacc, tile, mybir, bass
    import ml_dtypes

    dist = dist or DIST
    f32 = mybir.dt.float32
    f32r = mybir.dt.float32r
    bf16 = mybir.dt.bfloat16
    u32 = mybir.dt.uint32
    Alu = mybir.AluOpType

    nc = bacc.Bacc("TRN2", target_bir_lowering=False)

    if dist == "f32r2":
        z_ext = nc.declare_dram_parameter("z", [bpc, 128, 2, N], f32r,
                                          isOutput=False)
        ct_ext = nc.declare_dram_parameter("ct", [bpc, 128, 2, 2, K], f32r,
                                           isOutput=False)
    else:
        z1_ext = nc.declare_dram_parameter("z1", [bpc, 128, 2, N], bf16,
                                           isOutput=False)
        z2_ext = nc.declare_dram_parameter("z2", [bpc, 128, 2, N], bf16,
                                           isOutput=False)
        ct1_ext = nc.declare_dram_parameter("ct1", [bpc, 128, 2, K], bf16,
                                            isOutput=False)
        ct2_ext = nc.declare_dram_parameter("ct2", [bpc, 128, 2, K], bf16,
                                            isOutput=False)
    bias_ext = nc.declare_dram_parameter("bias", [bpc, 1, K], f32r,
                                         isOutput=False)
    ones_ext = nc.declare_dram_parameter("ones", [1, 128], f32r, isOutput=False)
    # gather table: per-batch class codebooks as bf16 rows
    tab_ext = nc.declare_dram_parameter("tab", [bpc * K, D], bf16,
                                        isOutput=False)
    out_ext = nc.declare_dram_parameter("out", [bpc, 128, nt, D], bf16,
                                        isOutput=True)

    iota = np.broadcast_to(np.arange(K, dtype=np.float32), (128, K))
    w_dram = nc.inline_tensor(np.ascontiguousarray(iota), name="iota")

    with tile.TileContext(nc) as tc:
        with (
            tc.tile_pool(name="const", bufs=1) as constp,
            tc.tile_pool(name="zb", bufs=zb_bufs) as zb,
            tc.tile_pool(name="cbp", bufs=2) as cbp,
            tc.tile_pool(name="outp", bufs=2) as outp,
            tc.tile_pool(name="sco", bufs=sco_bufs) as sco,
            tc.tile_pool(name="idxp", bufs=4) as idxp,
            tc.tile_pool(name="psS", bufs=psum_bufs, space="PSUM") as psSp,
        ):
            w = constp.tile([128, K], f32, tag="iota")
            ones = constp.tile([1, 128], f32r, tag="ones")
            nc.sync.dma_start(w[:], w_dram[:])
            nc.sync.dma_start(ones[:], ones_ext[:])

            for b in [bb for _ in range(repeat) for bb in range(bpc)]:
                bias = cbp.tile([1, K], f32r, tag="bias")
                nc.sync.dma_start(bias[:], bias_ext[b])
                if dist == "f32r2":
                    z = zb.tile([128, 2, N], f32r, tag="z")
                    ct = cbp.tile([128, 2, 2, K], f32r, tag="ct")
                    nc.sync.dma_start(ct[:], ct_ext[b])
                    nc.sync.dma_start(z[:], z_ext[b])
                    mms = [(z, (0, 0)), (z, (0, 1)), (z, (1, 0)), (z, (1, 1))]
                else:
                    z1 = zb.tile([128, 2, N], bf16, tag="z1")
                    z2 = zb.tile([128, 2, N], bf16, tag="z2")
                    ct1 = cbp.tile([128, 2, K], bf16, tag="ct1")
                    ct2 = cbp.tile([128, 2, K], bf16, tag="ct2")
                    nc.sync.dma_start(ct1[:], ct1_ext[b])
                    nc.sync.dma_start(ct2[:], ct2_ext[b])
                    nc.sync.dma_start(z1[:], z1_ext[b])
                    nc.sync.dma_start(z2[:], z2_ext[b])

                out_g = outp.tile([128, nt, D], bf16, tag="out")

                for p in range(nt // 2):
                    psS = psSp.tile([128, 2, K], f32, tag="psS")
                    for h in range(2):
                        n0 = (2 * p + h) * 128
                        # bias lands first (contraction-1 matmul), scores
                        # accumulate on top
                        nc.tensor.matmul(psS[:, h, :], ones[:], bias[:],
                                         start=True, stop=False)
                        if dist == "f32r2":
                            for i, (zz, (s, cd)) in enumerate(mms):
                                nc.tensor.matmul(
                                    psS[:, h, :], zz[:, cd, n0:n0 + 128],
                                    ct[:, s, cd, :], start=False,
                                    stop=(i == 3))
                        else:
                            six = [(z1, ct1, 0), (z1, ct1, 1), (z2, ct1, 0),
                                   (z2, ct1, 1), (z1, ct2, 0), (z1, ct2, 1)]
                            for i, (za, ca, cd) in enumerate(six):
                                nc.tensor.matmul(
                                    psS[:, h, :], za[:, cd, n0:n0 + 128],
                                    ca[:, cd, :], start=False, stop=(i == 5))

                    if tail == "nored":
                        nc.scalar.copy(out_g[:, 2 * p:2 * p + 2, :],
                                       psS[:, :, 0:D])
                        continue

                    # biased scores PSUM -> SBUF on ScalarE
                    S_sb = sco.tile([128, 2, K], f32, tag="S")
                    nc.scalar.copy(S_sb[:], psS[:])
                    mx = sco.tile([128, 2], f32, tag="mx")
                    nc.vector.reduce_max(mx[:], S_sb[:],
                                         axis=mybir.AxisListType.X)

                    if tail == "noidx":
                        nc.scalar.copy(out_g[:, 2 * p:2 * p + 2, :],
                                       S_sb[:, :, 0:D])
                        continue

                    # idx[n] = sum_k (S==mx)*k in ONE fused DVE op, then a
                    # tiny add+cast gives uint32 row offsets into the table
                    junk = sco.tile([128, 2, K], f32, tag="junk")
                    for h in range(2):
                        idxf = idxp.tile([128, 1], f32, tag=f"idxf{h}")
                        idxu = idxp.tile([128, 1], u32, tag=f"idxu{h}")
                        nc.vector.scalar_tensor_tensor(
                            out=junk[:, h, :], in0=S_sb[:, h, :],
                            scalar=mx[:, h:h + 1], in1=w[:],
                            op0=Alu.is_equal, op1=Alu.mult,
                            accum_out=idxf[:])
                        nc.vector.tensor_scalar(
                            out=idxu[:], in0=idxf[:],
                            scalar1=float(b * K), scalar2=None, op0=Alu.add)
                        if tail == "nodma":
                            continue
                        # gather the selected codebook rows from DRAM
                        nc.gpsimd.indirect_dma_start(
                            out=out_g[:, 2 * p + h, :], out_offset=None,
                            in_=tab_ext[:],
                            in_offset=bass.IndirectOffsetOnAxis(
                                ap=idxu[:], axis=0),
                        )
                    if tail == "nodma":
                        nc.scalar.copy(out_g[:, 2 * p:2 * p + 2, :],
                                       junk[:, :, 0:D])

                # store on the Act HWDGE ring so it never blocks SP loads
                nc.scalar.dma_start(out_ext[b], out_g[:])

    nc.compile()
    return nc


def _get_nc():
    if "nc" not in _CACHE:
        _CACHE["nc"] = _build()
    return _CACHE["nc"]


def _prep_in_maps(z_e_x, c, embedding, dist=None):
    import ml_dtypes

    dist = dist or DIST
    bf = ml_dtypes.bfloat16

    z = np.ascontiguousarray(np.asarray(z_e_x), dtype=np.float32)      # [B, D, H, W]
    cls = np.asarray(c).astype(np.int64)                               # [B]
    emb = np.ascontiguousarray(np.asarray(embedding), dtype=np.float32)

    zf = z.reshape(B, D, N)                                            # [B, 256, 4096]
    cb = emb.reshape(NUM_CLASSES, K, D)[cls]                           # [B, 512, 256]
    cbT = np.ascontiguousarray(cb.transpose(0, 2, 1))                  # [B, 256, 512]

    com = {}
    if dist == "f32r2":
        zq = np.ascontiguousarray(zf.reshape(B, 2, 128, N).transpose(0, 2, 1, 3))
        c1 = cbT.astype(bf).astype(np.float32)
        c2 = cbT - c1
        ct = np.stack([c1, c2], axis=1)                                # [B, 2, 256, K]
        ct = np.ascontiguousarray(
            ct.reshape(B, 2, 2, 128, K).transpose(0, 3, 1, 2, 4))      # [B,128,2,2,K]
        com.update(z=zq, ct=ct)
    else:
        z1 = zf.astype(bf)
        z2 = (zf - z1.astype(np.float32)).astype(bf)
        z1 = np.ascontiguousarray(z1.reshape(B, 2, 128, N).transpose(0, 2, 1, 3))
        z2 = np.ascontiguousarray(z2.reshape(B, 2, 128, N).transpose(0, 2, 1, 3))
        ct1 = cbT.astype(bf)
        ct2 = (cbT - ct1.astype(np.float32)).astype(bf)
        ct1 = np.ascontiguousarray(ct1.reshape(B, 2, 128, K).transpose(0, 2, 1, 3))
        ct2 = np.ascontiguousarray(ct2.reshape(B, 2, 128, K).transpose(0, 2, 1, 3))
        com.update(z1=z1, z2=z2, ct1=ct1, ct2=ct2)

    bias = (-0.5 * np.sum(cb.astype(np.float64) ** 2, axis=2)).astype(np.float32)
    bias = np.ascontiguousarray(bias[:, None, :])                      # [B, 1, K]
    ones = np.ones((1, 128), dtype=np.float32)
    tab = np.ascontiguousarray(cb.astype(bf))                          # [B, K, D]

    in_maps = []
    for i in range(NCORES):
        s = slice(i * BPC, (i + 1) * BPC)
        m = {k: v[s] for k, v in com.items()}
        m["bias"] = bias[s]
        m["ones"] = ones
        m["tab"] = tab[s].reshape(BPC * K, D)
        in_maps.append(m)
    return in_maps


def kernel(z_e_x, c, embedding):
    from concourse.bass_utils import run_bass_kernel_spmd

    global LAST_EXEC_NS

    in_maps = _prep_in_maps(z_e_x, c, embedding)
    nc = _get_nc()
    res = run_bass_kernel_spmd(nc, in_maps, core_ids=list(range(NCORES)),
                               trace=TRACE)
    LAST_EXEC_NS = res.exec_time_ns

    outs = np.concatenate([res.results[i]["out"].astype(np.float32)
                           for i in range(NCORES)], axis=0)
    # [B, 128, NT, D] -> [B, N, D] with n = t*128 + p
    out = outs.transpose(0, 2, 1, 3).reshape(B, N, D)
    return np.ascontiguousarray(out.reshape(B, HH, WW, D))


# revision 10
# speedup vs baseline: 1.3338x; 1.3338x over previous
"""VQ codebook-lookup kernel for TRN2, data-parallel over batch on 8 NeuronCores.

Reference computation (per batch b with class c[b]):
  z = z_e_x[b] viewed as [N=4096, D=256] (D innermost)
  cb = embedding[c[b]*512:(c[b]+1)*512]            # [K=512, D]
  idx[n] = argmin_k ||z[n] - cb[k]||^2 = argmax_k (z[n].cb[k] - ||cb[k]||^2/2)
  out[n] = cb[idx[n]]

Device strategy per core (4 batches), engines balanced:
  - scores S[n,k] on TensorE: 3-pass split-bf16 matmuls (z1c1+z2c1+z1c2,
    ~2^-16 dot error, exact argmax in practice); bias folded in as a
    contraction-1 ones x bias matmul so PSUM holds biased scores directly
    (fp32r variants measured ~2x slower per matmul on HW than bf16 and
    their ~1.5e-4 rounding costs ~20-30 argmax flips -- not worth it)
  - ScalarE moves biased scores PSUM->SBUF
  - VectorE: row-max, then ONE fused scalar_tensor_tensor computes
    idx[n] = sum_k (S==mx)*k, plus a tiny cast to uint32
  - GPSIMD issues an indirect DMA per n-tile that gathers the selected
    codebook rows (bf16) straight from DRAM -- no transpose, no one-hot
    matmul, no PSUM round-trip for the gather
Host side only reindexes/splits operands and reassembles the output.
"""

import sys

sys.path.insert(0, "/opt/trn_rl_repo")

import numpy as np

B, D, HH, WW = 32, 256, 64, 64
N = HH * WW            # 4096 positions per batch
K = 512                # codes per class
NUM_CLASSES = 60
NCORES = 8
BPC = B // NCORES      # batches per core
NT = N // 128          # 32 n-tiles per batch

DIST = "bf3"           # "f32r2" (fp32r c-split 2-pass) | "bf3" (split-bf16 3-pass)

_CACHE = {}

# set by test harness to request an NTFF profile
TRACE = False
LAST_EXEC_NS = None


def _build(bpc=BPC, nt=NT, repeat=1, dist=None, tail="full", psum_bufs=3,
           sco_bufs=3, zb_bufs=2):
    # tail: "full" | "nored" (PE only) | "noidx" (+Act copy, DVE max)
    #       | "nodma" (+stt idx extract; skip the indirect gather)
    from concourse import bacc, tile, mybir, bass
    import ml_dtypes

    dist = dist or DIST
    f32 = mybir.dt.float32
    f32r = mybir.dt.float32r
    bf16 = mybir.dt.bfloat16
    u32 = mybir.dt.uint32
    Alu = mybir.AluOpType

    nc = bacc.Bacc("TRN2", target_bir_lowering=False)

    if dist == "f32r2":
        z_ext = nc.declare_dram_parameter("z", [bpc, 128, 2, N], f32r,
                                          isOutput=False)
        ct_ext = nc.declare_dram_parameter("ct", [bpc, 128, 2, 2, K], f32r,
                                           isOutput=False)
    else:
        z1_ext = nc.declare_dram_parameter("z1", [bpc, 128, 2, N], bf16,
                                           isOutput=False)
        z2_ext = nc.declare_dram_parameter("z2", [bpc, 128, 2, N], bf16,
                                           isOutput=False)
        ct1_ext = nc.declare_dram_parameter("ct1", [bpc, 128, 2, K], bf16,
                                            isOutput=False)
        ct2_ext = nc.declare_dram_parameter("ct2", [bpc, 128, 2, K], bf16,
                                            isOutput=False)
    bias_ext = nc.declare_dram_parameter("bias", [bpc, 1, K], f32r,
                                         isOutput=False)
    ones_ext = nc.declare_dram_parameter("ones", [1, 128], f32r, isOutput=False)
    # gather table: per-batch class codebooks as bf16 rows
    tab_ext = nc.declare_dram_parameter("tab", [bpc * K, D], bf16,
                                        isOutput=False)
    out_ext = nc.declare_dram_parameter("out", [bpc, 128, nt, D], bf16,
                                        isOutput=True)

    iota = np.broadcast_to(np.arange(K, dtype=np.float32), (128, K))
    w_dram = nc.inline_tensor(np.ascontiguousarray(iota), name="iota")

    with tile.TileContext(nc) as tc:
        with (
            tc.tile_pool(name="const", bufs=1) as constp,
            tc.tile_pool(name="zb", bufs=zb_bufs) as zb,
            tc.tile_pool(name="cbp", bufs=2) as cbp,
            tc.tile_pool(name="outp", bufs=2) as outp,
            tc.tile_pool(name="sco", bufs=sco_bufs) as sco,
            tc.tile_pool(name="idxp", bufs=4) as idxp,
            tc.tile_pool(name="psS", bufs=psum_bufs, space="PSUM") as psSp,
        ):
            w = constp.tile([128, K], f32, tag="iota")
            ones = constp.tile([1, 128], f32r, tag="ones")
            nc.sync.dma_start(w[:], w_dram[:])
            nc.sync.dma_start(ones[:], ones_ext[:])

            for b in [bb for _ in range(repeat) for bb in range(bpc)]:
                bias = cbp.tile([1, K], f32r, tag="bias")
                nc.sync.dma_start(bias[:], bias_ext[b])
                if dist == "f32r2":
                    z = zb.tile([128, 2, N], f32r, tag="z")
                    ct = cbp.tile([128, 2, 2, K], f32r, tag="ct")
                    nc.sync.dma_start(ct[:], ct_ext[b])
                    nc.sync.dma_start(z[:], z_ext[b])
                    mms = [(z, (0, 0)), (z, (0, 1)), (z, (1, 0)), (z, (1, 1))]
                else:
                    z1 = zb.tile([128, 2, N], bf16, tag="z1")
                    z2 = zb.tile([128, 2, N], bf16, tag="z2")
                    ct1 = cbp.tile([128, 2, K], bf16, tag="ct1")
                    ct2 = cbp.tile([128, 2, K], bf16, tag="ct2")
                    nc.sync.dma_start(ct1[:], ct1_ext[b])
                    nc.sync.dma_start(ct2[:], ct2_ext[b])
                    nc.sync.dma_start(z1[:], z1_ext[b])
                    nc.sync.dma_start(z2[:], z2_ext[b])

                out_g = outp.tile([128, nt, D], bf16, tag="out")

                for p in range(nt // 2):
                    psS = psSp.tile([128, 2, K], f32, tag="psS")
                    for h in range(2):
                        n0 = (2 * p + h) * 128
                        # bias lands first (contraction-1 matmul), scores
                        # accumulate on top
                        nc.tensor.matmul(psS[:, h, :], ones[:], bias[:],
                                         start=True, stop=False)
                        if dist == "f32r2":
                            for i, (zz, (s, cd)) in enumerate(mms):
                                nc.tensor.matmul(
                                    psS[:, h, :], zz[:, cd, n0:n0 + 128],
                                    ct[:, s, cd, :], start=False,
                                    stop=(i == 3))
                        else:
                            six = [(z1, ct1, 0), (z1, ct1, 1), (z2, ct1, 0),
                                   (z2, ct1, 1), (z1, ct2, 0), (z1, ct2, 1)]
                            for i, (za, ca, cd) in enumerate(six):
                                nc.tensor.matmul(
                                    psS[:, h, :], za[:, cd, n0:n0 + 128],
                                    ca[:, cd, :], start=False, stop=(i == 5))

                    if tail == "nored":
                        nc.scalar.copy(out_g[:, 2 * p:2 * p + 2, :],
                                       psS[:, :, 0:D])
                        continue

                    # biased scores PSUM -> SBUF on ScalarE
                    S_sb = sco.tile([128, 2, K], f32, tag="S")
                    nc.scalar.copy(S_sb[:], psS[:])
                    mx = sco.tile([128, 2], f32, tag="mx")
                    nc.vector.reduce_max(mx[:], S_sb[:],
                                         axis=mybir.AxisListType.X)

                    if tail == "noidx":
                        nc.scalar.copy(out_g[:, 2 * p:2 * p + 2, :],
                                       S_sb[:, :, 0:D])
                        continue

                    # idx[n] = sum_k (S==mx)*k in ONE fused DVE op, then a
                    # tiny add+cast gives uint32 row offsets into the table
                    junk = sco.tile([128, 2, K], f32, tag="junk")
                    for h in range(2):
                        idxf = idxp.tile([128, 1], f32, tag=f"idxf{h}")
                        idxu = idxp.tile([128, 1], u32, tag=f"idxu{h}")
                        nc.vector.scalar_tensor_tensor(
                            out=junk[:, h, :], in0=S_sb[:, h, :],
                            scalar=mx[:, h:h + 1], in1=w[:],
                            op0=Alu.is_equal, op1=Alu.mult,
                            accum_out=idxf[:])
                        nc.vector.tensor_scalar(
                            out=idxu[:], in0=idxf[:],
                            scalar1=float(b * K), scalar2=None, op0=Alu.add)
                        if tail == "nodma":
                            continue
                        # gather the selected codebook rows from DRAM
                        nc.gpsimd.indirect_dma_start(
                            out=out_g[:, 2 * p + h, :], out_offset=None,
                            in_=tab_ext[:],
                            in_offset=bass.IndirectOffsetOnAxis(
                                ap=idxu[:], axis=0),
                        )
                    if tail == "nodma":
                        nc.scalar.copy(out_g[:, 2 * p:2 * p + 2, :],
                                       junk[:, :, 0:D])

                # store on the Act HWDGE ring so it never blocks SP loads
                nc.scalar.dma_start(out_ext[b], out_g[:])

    nc.compile()
    return nc


def _get_nc():
    if "nc" not in _CACHE:
        _CACHE["nc"] = _build()
    return _CACHE["nc"]


def _prep_in_maps(z_e_x, c, embedding, dist=None):
    import ml_dtypes

    dist = dist or DIST
    bf = ml_dtypes.bfloat16

    z = np.ascontiguousarray(np.asarray(z_e_x), dtype=np.float32)      # [B, D, H, W]
    cls = np.asarray(c).astype(np.int64)                               # [B]
    emb = np.ascontiguousarray(np.asarray(embedding), dtype=np.float32)

    zf = z.reshape(B, D, N)                                            # [B, 256, 4096]
    cb = emb.reshape(NUM_CLASSES, K, D)[cls]                           # [B, 512, 256]
    cbT = np.ascontiguousarray(cb.transpose(0, 2, 1))                  # [B, 256, 512]

    com = {}
    if dist == "f32r2":
        zq = np.ascontiguousarray(zf.reshape(B, 2, 128, N).transpose(0, 2, 1, 3))
        c1 = cbT.astype(bf).astype(np.float32)
        c2 = cbT - c1
        ct = np.stack([c1, c2], axis=1)                                # [B, 2, 256, K]
        ct = np.ascontiguousarray(
            ct.reshape(B, 2, 2, 128, K).transpose(0, 3, 1, 2, 4))      # [B,128,2,2,K]
        com.update(z=zq, ct=ct)
    else:
        z1 = zf.astype(bf)
        z2 = (zf - z1.astype(np.float32)).astype(bf)
        z1 = np.ascontiguousarray(z1.reshape(B, 2, 128, N).transpose(0, 2, 1, 3))
        z2 = np.ascontiguousarray(z2.reshape(B, 2, 128, N).transpose(0, 2, 1, 3))
        ct1 = cbT.astype(bf)
        ct2 = (cbT - ct1.astype(np.float32)).astype(bf)
        ct1 = np.ascontiguousarray(ct1.reshape(B, 2, 128, K).transpose(0, 2, 1, 3))
        ct2 = np.ascontiguousarray(ct2.reshape(B, 2, 128, K).transpose(0, 2, 1, 3))
        com.update(z1=z1, z2=z2, ct1=ct1, ct2=ct2)

    bias = (-0.5 * np.sum(cb.astype(np.float64) ** 2, axis=2)).astype(np.float32)
    bias = np.ascontiguousarray(bias[:, None, :])                      # [B, 1, K]
    ones = np.ones((1, 128), dtype=np.float32)
    tab = np.ascontiguousarray(cb.astype(bf))                          # [B, K, D]

    in_maps = []
    for i in range(NCORES):
        s = slice(i * BPC, (i + 1) * BPC)
        m = {k: v[s] for k, v in com.items()}
        m["bias"] = bias[s]
        m["ones"] = ones
        m["tab"] = tab[s].reshape(BPC * K, D)
        in_maps.append(m)
    return in_maps


def kernel(z_e_x, c, embedding):
    from concourse.bass_utils import run_bass_kernel_spmd

    global LAST_EXEC_NS

    in_maps = _prep_in_maps(z_e_x, c, embedding)
    nc = _get_nc()
    res = run_bass_kernel_spmd(nc, in_maps, core_ids=list(range(NCORES)),
                               trace=TRACE)
    LAST_EXEC_NS = res.exec_time_ns

    outs = np.concatenate([res.results[i]["out"].astype(np.float32)
                           for i in range(NCORES)], axis=0)
    # [B, 128, NT, D] -> [B, N, D] with n = t*128 + p
    out = outs.transpose(0, 2, 1, 3).reshape(B, N, D)
    return np.ascontiguousarray(out.reshape(B, HH, WW, D))
